# revision 13
# baseline (speedup 1.0000x reference)
"""ButterflyMlp Trainium2 kernel.

Reference computation (B=65536):
    h1 = relu(x @ (W1*m1).T + b1)          # [B, 784]
    h2 = relu(h1 @ (W2*m2).T + b2)         # [B, 128]
    logits = h2 @ (W3*m3).T + b3           # [B, 10]
    out = log_softmax(logits, axis=1)

Strategy: pure data parallel over 8 NeuronCores (batch sharded 8192/core,
masked weights replicated).  Activations are kept in transposed
[features, batch] layout on-chip so every layer contracts over the SBUF
partition dimension with the weight tile stationary.  The whole per-core
x shard (8 MB fp8) is DMA'd into SBUF up front — DMA instruction issue
on the queue engine costs ~0.6 us each, so few big transfers beat many
small ones.

Layers 1 and 2 run in fp8e4m3 with DoubleRow perf mode (2 fp8 weights
per PE cell -> K=256 contraction per matmul) and fp32 PSUM accumulation.
The masked weights are pre-scaled by 32 (h1 stored at the same x32
scale) to keep fp8 values in the normal range; the scales are folded
back in the relu evacuations, which alternate between the Scalar and
Vector engines to split the load.  Layer 3 + log_softmax run in
bf16/fp32.  End-to-end max relative error vs the fp32 reference is
~3e-4.
"""

import numpy as np
import ml_dtypes

import concourse.bass as bass
import concourse.mybir as mybir
import concourse.tile as tile
from concourse import bacc
from concourse.bass_utils import run_bass_kernel_spmd

BF16 = ml_dtypes.bfloat16
FP8 = ml_dtypes.float8_e4m3
F32 = np.float32

N_CORES = 8
B = 65536
S = B // N_CORES          # batch rows per core
IN_F = 784
KT1 = 7                   # k-tiles for layer-1 contraction (x padded 784->896)
KT2 = 7                   # k-tiles for layer-2 contraction (h1 padded 784->896)
PAD2 = KT2 * 128
H2 = 128
NCLS = 10
NSUB = S // 512           # 512-wide matmul sub-blocks per core
NT3 = S // 128            # 128-row batch tiles per core (layer 3)
NSMX = 16                 # layer-3 batch tiles per softmax group
NGRP = NT3 // NSMX        # softmax groups

SW = 32.0                 # fp8 weight pre-scale (W1, W2); h1 is stored at scale SW

WINDOW, STRIPES, STEP = 10, 5, 3

_CACHE = {}


def _butterfly_mask(out_f, in_f, window=WINDOW, stripes=STRIPES, step=STEP):
    i = np.arange(out_f)[:, None]
    j = np.arange(in_f)[None, :]
    jc = (i * in_f) // out_f
    band = np.abs(j - jc) <= window
    period = max(in_f // stripes, 1)
    stripe = ((j - jc) % period) < step
    return (band | stripe).astype(np.float32)


def _build_nc():
    nc = bacc.Bacc("TRN2", target_bir_lowering=False, debug=False, num_devices=N_CORES)

    # host-side layouts are pre-rearranged so every DMA is contiguous per
    # partition:  xq [KT1, 128, S],  w1q [128, ot*kt*oi],  bias pack [128, 18]
    xq = nc.dram_tensor("xq", [KT1, 128, S], mybir.dt.float8e4, kind="ExternalInput")
    w1q = nc.dram_tensor("w1q", [128, KT2 * KT1 * 128], mybir.dt.float8e4, kind="ExternalInput")
    w2q = nc.dram_tensor("w2q", [128, KT2 * H2], mybir.dt.float8e4, kind="ExternalInput")
    w3q = nc.dram_tensor("w3q", [H2, NCLS], mybir.dt.bfloat16, kind="ExternalInput")
    bias = nc.dram_tensor("bias", [128, KT2 + 1 + NCLS], mybir.dt.float32, kind="ExternalInput")
    out = nc.dram_tensor("out", [S, NCLS], mybir.dt.float32, kind="ExternalOutput")

    Relu = mybir.ActivationFunctionType.Relu
    Exp = mybir.ActivationFunctionType.Exp
    Ln = mybir.ActivationFunctionType.Ln
    X = mybir.AxisListType.X
    DR = mybir.MatmulPerfMode.DoubleRow

    with tile.TileContext(nc) as tc:
        with (
            tc.tile_pool(name="consts", bufs=1) as consts,
            tc.tile_pool(name="spool", bufs=2) as spool,
            tc.tile_pool(name="opool", bufs=2) as opool,
            tc.tile_pool(name="ps1", bufs=4, space="PSUM") as ps1,
            tc.tile_pool(name="ps2", bufs=2, space="PSUM") as ps2,
            tc.tile_pool(name="ps3", bufs=2, space="PSUM") as ps3,
        ):
            # w1 SBUF layout [p, o_tile, kt, oi]; single contiguous DMA
            w1_sb = consts.tile([128, KT2, KT1, 128], mybir.dt.float8e4)
            nc.sync.dma_start(
                w1_sb[:], w1q.rearrange("p (ot kt oi) -> p ot kt oi", ot=KT2, kt=KT1)
            )

            # whole x shard in SBUF: [128, kt, S] fp8 = 56 KB/partition
            xt_all = consts.tile([128, KT1, S], mybir.dt.float8e4)
            for k in range(KT1):
                nc.sync.dma_start(xt_all[:, k, :], xq[k, :, :])

            w2_sb = consts.tile([128, KT2, H2], mybir.dt.float8e4)
            nc.sync.dma_start(w2_sb[:], w2q.rearrange("p (kt o) -> p kt o", kt=KT2))
            w3_sb = consts.tile([128, NCLS], mybir.dt.bfloat16)
            nc.sync.dma_start(w3_sb[:], w3q[:, :])
            bias_sb = consts.tile([128, KT2 + 1 + NCLS], mybir.dt.float32)
            nc.sync.dma_start(bias_sb[:], bias[:, :])
            b1_sb = bias_sb[:, 0:KT2]
            b2_sb = bias_sb[:, KT2 : KT2 + 1]
            b3_sb = bias_sb[:, KT2 + 1 :]

            # persistent whole-shard activations
            h1_all = consts.tile([128, KT2, S], mybir.dt.float8e4)
            h2_all = consts.tile([128, S], mybir.dt.bfloat16)

            # ---- layer 1 (fp8 DoubleRow): h1T = relu(W1mT.T @ xT + b1) ----
            for o in range(KT2):
                for nb in range(NSUB):
                    ns = slice(nb * 512, (nb + 1) * 512)
                    ps = ps1.tile([128, 512], mybir.dt.float32, tag="ps1")
                    for p in range(3):
                        nc.tensor.matmul(
                            ps[:],
                            w1_sb[:, o, 2 * p : 2 * p + 2, :],
                            xt_all[:, 2 * p : 2 * p + 2, ns],
                            start=(p == 0),
                            stop=False,
                            perf_mode=DR,
                        )
                    nc.tensor.matmul(
                        ps[:],
                        w1_sb[:, o, KT1 - 1, :],
                        xt_all[:, KT1 - 1, ns],
                        start=False,
                        stop=True,
                    )
                    # psum = SW * (x @ W1m.T) ; h1 stored = relu(psum + SW*b1)
                    # = SW * relu(true + b1).  Evacuation alternates between
                    # the Scalar and Vector engines to split the load.
                    h1_dst = h1_all[:, o, ns]
                    if o % 2 == 0:
                        nc.vector.tensor_scalar(
                            h1_dst,
                            ps[:],
                            b1_sb[:, o : o + 1],
                            0.0,
                            mybir.AluOpType.add,
                            mybir.AluOpType.max,
                        )
                    else:
                        nc.scalar.activation(
                            h1_dst, ps[:], Relu, bias=b1_sb[:, o : o + 1], scale=1.0
                        )

            # ---- layer 2 (fp8 DoubleRow + tail): h2T = relu(W2mT.T @ h1T + b2) ----
            for nb in range(NSUB):
                ns = slice(nb * 512, (nb + 1) * 512)
                ps = ps2.tile([128, 512], mybir.dt.float32, tag="ps2")
                for p in range(3):
                    nc.tensor.matmul(
                        ps[:],
                        w2_sb[:, 2 * p : 2 * p + 2, :],
                        h1_all[:, 2 * p : 2 * p + 2, ns],
                        start=(p == 0),
                        stop=False,
                        perf_mode=DR,
                    )
                nc.tensor.matmul(
                    ps[:],
                    w2_sb[:, KT2 - 1, :],
                    h1_all[:, KT2 - 1, ns],
                    start=False,
                    stop=True,
                )
                # psum = SW * SW * (h1 @ W2m.T)
                nc.scalar.activation(
                    h2_all[:, ns],
                    ps[:],
                    Relu,
                    bias=b2_sb[:, 0:1],
                    scale=1.0 / (SW * SW),
                )

            # ---- layer 3 (bf16): logits[b, c] then log_softmax along c ----
            for g in range(NGRP):
                ps_l = ps3.tile([128, NSMX, NCLS], mybir.dt.float32, tag="ps3")
                for bt in range(NSMX):
                    bt_abs = g * NSMX + bt
                    nc.tensor.matmul(
                        ps_l[:, bt, :],
                        h2_all[:, bt_abs * 128 : (bt_abs + 1) * 128],
                        w3_sb[:, :],
                        start=(bt == 0),
                        stop=(bt == NSMX - 1),
                        skip_group_check=True,
                    )

                z = spool.tile([128, NSMX, NCLS], mybir.dt.float32, tag="z")
                nc.vector.tensor_add(
                    z[:], ps_l[:], b3_sb[:, None, :].to_broadcast((128, NSMX, NCLS))
                )
                zm = spool.tile([128, NSMX], mybir.dt.float32, tag="zm")
                nc.vector.reduce_max(zm[:], z[:], axis=X)
                zs = spool.tile([128, NSMX, NCLS], mybir.dt.float32, tag="zs")
                nc.vector.tensor_sub(
                    zs[:], z[:], zm[:, :, None].to_broadcast((128, NSMX, NCLS))
                )
                e = spool.tile([128, NSMX, NCLS], mybir.dt.float32, tag="e")
                nc.scalar.activation(e[:], zs[:], Exp)
                se = spool.tile([128, NSMX], mybir.dt.float32, tag="se")
                nc.vector.reduce_sum(se[:], e[:], axis=X)
                lse = spool.tile([128, NSMX], mybir.dt.float32, tag="lse")
                nc.scalar.activation(lse[:], se[:], Ln)
                ot = opool.tile([128, NSMX, NCLS], mybir.dt.float32, tag="ot")
                nc.vector.tensor_sub(
                    ot[:], zs[:], lse[:, :, None].to_broadcast((128, NSMX, NCLS))
                )
                nc.sync.dma_start(
                    out[g * NSMX * 128 : (g + 1) * NSMX * 128, :].rearrange(
                        "(bt p) c -> p bt c", p=128
                    ),
                    ot[:],
                )

    return nc


def _prep_inputs(x, W1, b1, W2, b2, W3, b3):
    m1 = _butterfly_mask(IN_F, IN_F)
    m2 = _butterfly_mask(H2, IN_F)
    m3 = _butterfly_mask(NCLS, H2)

    # w1: [in(pad 896), out(pad 896)] scaled by SW, laid out [p, ot, kt, oi]
    w1t = np.zeros((KT1 * 128, PAD2), dtype=F32)
    w1t[:IN_F, :IN_F] = (np.asarray(W1, F32) * m1).T * SW
    w1l = np.ascontiguousarray(
        w1t.reshape(KT1, 128, KT2, 128)
        .transpose(1, 2, 0, 3)
        .reshape(128, KT2 * KT1 * 128)
    ).astype(FP8)

    w2t = np.zeros((PAD2, H2), dtype=F32)
    w2t[:IN_F, :] = (np.asarray(W2, F32) * m2).T * SW
    w2l = np.ascontiguousarray(
        w2t.reshape(KT2, 128, H2).transpose(1, 0, 2).reshape(128, KT2 * H2)
    ).astype(FP8)

    w3l = ((np.asarray(W3, F32) * m3).T).astype(BF16).copy()

    # bias pack [128, 7 + 1 + 10] f32: b1 (x SW, per-partition by o-tile), b2, b3
    b1p = np.zeros((PAD2,), F32)
    b1p[:IN_F] = np.asarray(b1, F32) * SW
    bias = np.zeros((128, KT2 + 1 + NCLS), F32)
    bias[:, 0:KT2] = b1p.reshape(KT2, 128).T
    bias[:, KT2] = np.asarray(b2, F32)
    bias[:, KT2 + 1 :] = np.asarray(b3, F32)[None, :]
    bias = np.ascontiguousarray(bias)

    # x: [B, 784] -> fp8 -> padded transposed [KT1, 128, B]
    xp = np.zeros((KT1 * 128, B), dtype=FP8)
    xp[:IN_F, :] = np.asarray(x, F32).T.astype(FP8)
    xp = xp.reshape(KT1, 128, B)

    in_maps = []
    for c in range(N_CORES):
        in_maps.append(
            {
                "xq": np.ascontiguousarray(xp[:, :, c * S : (c + 1) * S]),
                "w1q": w1l,
                "w2q": w2l,
                "w3q": w3l,
                "bias": bias,
            }
        )
    return in_maps


def _run(inputs, trace=False, **run_kwargs):
    if "nc" not in _CACHE:
        nc = _build_nc()
        nc.finalize()
        _CACHE["nc"] = nc
    nc = _CACHE["nc"]
    in_maps = _prep_inputs(**inputs)
    res = run_bass_kernel_spmd(
        nc,
        in_maps,
        core_ids=list(range(N_CORES)),
        trace=trace,
        **run_kwargs,
    )
    out = np.concatenate([r["out"] for r in res.results], axis=0)
    return out, res


def kernel(**inputs):
    out, _ = _run(inputs, trace=False)
    return out


# revision 15
# speedup vs baseline: 1.1496x; 1.1496x over previous
"""ButterflyMlp Trainium2 kernel.

Reference computation (B=65536):
    h1 = relu(x @ (W1*m1).T + b1)          # [B, 784]
    h2 = relu(h1 @ (W2*m2).T + b2)         # [B, 128]
    logits = h2 @ (W3*m3).T + b3           # [B, 10]
    out = log_softmax(logits, axis=1)

Strategy: pure data parallel over 8 NeuronCores (batch sharded 8192/core,
masked weights replicated).  Activations are kept in transposed
[features, batch] layout on-chip so every layer contracts over the SBUF
partition dimension with the weight tile stationary.  The whole per-core
x shard (8 MB fp8) is DMA'd into SBUF up front — DMA instruction issue
on the queue engine costs ~0.6 us each, so few big transfers beat many
small ones.

Layers 1 and 2 run in fp8e4m3 with DoubleRow perf mode (2 fp8 weights
per PE cell -> K=256 contraction per matmul) and fp32 PSUM accumulation.
The masked weights are pre-scaled by 32 (h1 stored at the same x32
scale) to keep fp8 values in the normal range; the scales are folded
back in the relu evacuations, which alternate between the Scalar and
Vector engines to split the load.  Layer 3 + log_softmax run in
bf16/fp32.  End-to-end max relative error vs the fp32 reference is
~3e-4.
"""

import numpy as np
import ml_dtypes

import concourse.bass as bass
import concourse.mybir as mybir
import concourse.tile as tile
from concourse import bacc
from concourse.bass_utils import run_bass_kernel_spmd

BF16 = ml_dtypes.bfloat16
FP8 = ml_dtypes.float8_e4m3
F32 = np.float32

N_CORES = 8
B = 65536
S = B // N_CORES          # batch rows per core
IN_F = 784
KT1 = 7                   # k-tiles for layer-1 contraction (x padded 784->896)
KT2 = 7                   # k-tiles for layer-2 contraction (h1 padded 784->896)
PAD2 = KT2 * 128
H2 = 128
NCLS = 10
NSUB = S // 512           # 512-wide matmul sub-blocks per core
NT3 = S // 128            # 128-row batch tiles per core (layer 3)
NSMX = 16                 # layer-3 batch tiles per softmax group
NGRP = NT3 // NSMX        # softmax groups

SW = 32.0                 # fp8 weight pre-scale (W1, W2); h1 is stored at scale SW

WINDOW, STRIPES, STEP = 10, 5, 3

_CACHE = {}


def _butterfly_mask(out_f, in_f, window=WINDOW, stripes=STRIPES, step=STEP):
    i = np.arange(out_f)[:, None]
    j = np.arange(in_f)[None, :]
    jc = (i * in_f) // out_f
    band = np.abs(j - jc) <= window
    period = max(in_f // stripes, 1)
    stripe = ((j - jc) % period) < step
    return (band | stripe).astype(np.float32)


def _build_nc():
    nc = bacc.Bacc("TRN2", target_bir_lowering=False, debug=False, num_devices=N_CORES)

    # host-side layouts are pre-rearranged so every DMA is contiguous per
    # partition:  xq [KT1, 128, S],  w1q [128, ot*kt*oi],  bias pack [128, 18]
    xq = nc.dram_tensor("xq", [KT1, 128, S], mybir.dt.float8e4, kind="ExternalInput")
    w1q = nc.dram_tensor("w1q", [128, KT2 * KT1 * 128], mybir.dt.float8e4, kind="ExternalInput")
    w2q = nc.dram_tensor("w2q", [128, KT2 * H2], mybir.dt.float8e4, kind="ExternalInput")
    w3q = nc.dram_tensor("w3q", [H2, NCLS], mybir.dt.bfloat16, kind="ExternalInput")
    bias = nc.dram_tensor("bias", [128, KT2 + 1 + NCLS], mybir.dt.float32, kind="ExternalInput")
    out = nc.dram_tensor("out", [S, NCLS], mybir.dt.float32, kind="ExternalOutput")

    Relu = mybir.ActivationFunctionType.Relu
    Exp = mybir.ActivationFunctionType.Exp
    Ln = mybir.ActivationFunctionType.Ln
    X = mybir.AxisListType.X
    DR = mybir.MatmulPerfMode.DoubleRow

    with tile.TileContext(nc) as tc:
        with (
            tc.tile_pool(name="consts", bufs=1) as consts,
            tc.tile_pool(name="spool", bufs=2) as spool,
            tc.tile_pool(name="ps1", bufs=4, space="PSUM") as ps1,
            tc.tile_pool(name="ps2", bufs=2, space="PSUM") as ps2,
            tc.tile_pool(name="ps3", bufs=2, space="PSUM") as ps3,
        ):
            # w1 SBUF layout [p, o_tile, kt, oi]; single contiguous DMA
            w1_sb = consts.tile([128, KT2, KT1, 128], mybir.dt.float8e4)
            nc.sync.dma_start(
                w1_sb[:], w1q.rearrange("p (ot kt oi) -> p ot kt oi", ot=KT2, kt=KT1)
            )

            # whole x shard in SBUF: [128, kt, S] fp8 = 56 KB/partition.
            # DMA'd in 4 batch-column blocks so the first block's compute can
            # start while later blocks stream in.
            BLKC = S // NGRP
            xt_all = consts.tile([128, KT1, S], mybir.dt.float8e4)
            for k in range(KT1):
                nc.sync.dma_start(
                    xt_all[:, k, 0:BLKC], xq[k, :, 0:BLKC]
                )

            w2_sb = consts.tile([128, KT2, H2], mybir.dt.float8e4)
            nc.sync.dma_start(w2_sb[:], w2q.rearrange("p (kt o) -> p kt o", kt=KT2))
            w3_sb = consts.tile([128, NCLS], mybir.dt.bfloat16)
            nc.sync.dma_start(w3_sb[:], w3q[:, :])
            bias_sb = consts.tile([128, KT2 + 1 + NCLS], mybir.dt.float32)
            nc.sync.dma_start(bias_sb[:], bias[:, :])
            b1_sb = bias_sb[:, 0:KT2]
            b2_sb = bias_sb[:, KT2 : KT2 + 1]
            b3_sb = bias_sb[:, KT2 + 1 :]

            for g in range(1, NGRP):
                gs = slice(g * BLKC, (g + 1) * BLKC)
                for k in range(KT1):
                    nc.sync.dma_start(xt_all[:, k, gs], xq[k, :, gs])

            # persistent whole-shard activations
            h1_all = consts.tile([128, KT2, S], mybir.dt.float8e4)
            h2_all = consts.tile([128, S], mybir.dt.bfloat16)

            for g in range(NGRP):
                # ---- layer 1 (fp8 DoubleRow): h1T = relu(W1mT.T @ xT + b1) ----
                for o in range(KT2):
                    for nbl in range(BLKC // 512):
                        nb = g * (BLKC // 512) + nbl
                        ns = slice(nb * 512, (nb + 1) * 512)
                        ps = ps1.tile([128, 512], mybir.dt.float32, tag="ps1")
                        for p in range(3):
                            nc.tensor.matmul(
                                ps[:],
                                w1_sb[:, o, 2 * p : 2 * p + 2, :],
                                xt_all[:, 2 * p : 2 * p + 2, ns],
                                start=(p == 0),
                                stop=False,
                                perf_mode=DR,
                            )
                        nc.tensor.matmul(
                            ps[:],
                            w1_sb[:, o, KT1 - 1, :],
                            xt_all[:, KT1 - 1, ns],
                            start=False,
                            stop=True,
                        )
                        # psum = SW * (x @ W1m.T); h1 stored = relu(psum + SW*b1)
                        # = SW * relu(true + b1).  Evacuation alternates between
                        # the Scalar and Vector engines to split the load.
                        h1_dst = h1_all[:, o, ns]
                        if o % 2 == 0:
                            nc.vector.tensor_scalar(
                                h1_dst,
                                ps[:],
                                b1_sb[:, o : o + 1],
                                0.0,
                                mybir.AluOpType.add,
                                mybir.AluOpType.max,
                            )
                        else:
                            nc.scalar.activation(
                                h1_dst, ps[:], Relu, bias=b1_sb[:, o : o + 1], scale=1.0
                            )

                # ---- layer 2 (fp8 DoubleRow + tail): h2T = relu(W2mT.T @ h1T + b2) ----
                for nbl in range(BLKC // 512):
                    nb = g * (BLKC // 512) + nbl
                    ns = slice(nb * 512, (nb + 1) * 512)
                    ps = ps2.tile([128, 512], mybir.dt.float32, tag="ps2")
                    for p in range(3):
                        nc.tensor.matmul(
                            ps[:],
                            w2_sb[:, 2 * p : 2 * p + 2, :],
                            h1_all[:, 2 * p : 2 * p + 2, ns],
                            start=(p == 0),
                            stop=False,
                            perf_mode=DR,
                        )
                    nc.tensor.matmul(
                        ps[:],
                        w2_sb[:, KT2 - 1, :],
                        h1_all[:, KT2 - 1, ns],
                        start=False,
                        stop=True,
                    )
                    # psum = SW * SW * (h1 @ W2m.T)
                    nc.scalar.activation(
                        h2_all[:, ns],
                        ps[:],
                        Relu,
                        bias=b2_sb[:, 0:1],
                        scale=1.0 / (SW * SW),
                    )

                # ---- layer 3 (bf16): logits[b, c] then log_softmax along c ----
                ps_l = ps3.tile([128, NSMX, NCLS], mybir.dt.float32, tag="ps3")
                for bt in range(NSMX):
                    bt_abs = g * NSMX + bt
                    nc.tensor.matmul(
                        ps_l[:, bt, :],
                        h2_all[:, bt_abs * 128 : (bt_abs + 1) * 128],
                        w3_sb[:, :],
                        start=(bt == 0),
                        stop=(bt == NSMX - 1),
                        skip_group_check=True,
                    )

                z = spool.tile([128, NSMX, NCLS], mybir.dt.float32, tag="z")
                nc.vector.tensor_add(
                    z[:], ps_l[:], b3_sb[:, None, :].to_broadcast((128, NSMX, NCLS))
                )
                zm = spool.tile([128, NSMX], mybir.dt.float32, tag="zm")
                nc.vector.reduce_max(zm[:], z[:], axis=X)
                nc.vector.tensor_sub(
                    z[:], z[:], zm[:, :, None].to_broadcast((128, NSMX, NCLS))
                )
                e = spool.tile([128, NSMX, NCLS], mybir.dt.float32, tag="e")
                nc.scalar.activation(e[:], z[:], Exp)
                se = spool.tile([128, NSMX], mybir.dt.float32, tag="se")
                nc.vector.reduce_sum(se[:], e[:], axis=X)
                lse = spool.tile([128, NSMX], mybir.dt.float32, tag="lse")
                nc.scalar.activation(lse[:], se[:], Ln)
                nc.vector.tensor_sub(
                    e[:], z[:], lse[:, :, None].to_broadcast((128, NSMX, NCLS))
                )
                nc.sync.dma_start(
                    out[g * NSMX * 128 : (g + 1) * NSMX * 128, :].rearrange(
                        "(bt p) c -> p bt c", p=128
                    ),
                    e[:],
                )

    return nc


def _prep_inputs(x, W1, b1, W2, b2, W3, b3):
    m1 = _butterfly_mask(IN_F, IN_F)
    m2 = _butterfly_mask(H2, IN_F)
    m3 = _butterfly_mask(NCLS, H2)

    # w1: [in(pad 896), out(pad 896)] scaled by SW, laid out [p, ot, kt, oi]
    w1t = np.zeros((KT1 * 128, PAD2), dtype=F32)
    w1t[:IN_F, :IN_F] = (np.asarray(W1, F32) * m1).T * SW
    w1l = np.ascontiguousarray(
        w1t.reshape(KT1, 128, KT2, 128)
        .transpose(1, 2, 0, 3)
        .reshape(128, KT2 * KT1 * 128)
    ).astype(FP8)

    w2t = np.zeros((PAD2, H2), dtype=F32)
    w2t[:IN_F, :] = (np.asarray(W2, F32) * m2).T * SW
    w2l = np.ascontiguousarray(
        w2t.reshape(KT2, 128, H2).transpose(1, 0, 2).reshape(128, KT2 * H2)
    ).astype(FP8)

    w3l = ((np.asarray(W3, F32) * m3).T).astype(BF16).copy()

    # bias pack [128, 7 + 1 + 10] f32: b1 (x SW, per-partition by o-tile), b2, b3
    b1p = np.zeros((PAD2,), F32)
    b1p[:IN_F] = np.asarray(b1, F32) * SW
    bias = np.zeros((128, KT2 + 1 + NCLS), F32)
    bias[:, 0:KT2] = b1p.reshape(KT2, 128).T
    bias[:, KT2] = np.asarray(b2, F32)
    bias[:, KT2 + 1 :] = np.asarray(b3, F32)[None, :]
    bias = np.ascontiguousarray(bias)

    # x: [B, 784] -> fp8 -> padded transposed [KT1, 128, B]
    xp = np.zeros((KT1 * 128, B), dtype=FP8)
    xp[:IN_F, :] = np.asarray(x, F32).T.astype(FP8)
    xp = xp.reshape(KT1, 128, B)

    in_maps = []
    for c in range(N_CORES):
        in_maps.append(
            {
                "xq": np.ascontiguousarray(xp[:, :, c * S : (c + 1) * S]),
                "w1q": w1l,
                "w2q": w2l,
                "w3q": w3l,
                "bias": bias,
            }
        )
    return in_maps


def _run(inputs, trace=False, **run_kwargs):
    if "nc" not in _CACHE:
        nc = _build_nc()
        nc.finalize()
        _CACHE["nc"] = nc
    nc = _CACHE["nc"]
    in_maps = _prep_inputs(**inputs)
    res = run_bass_kernel_spmd(
        nc,
        in_maps,
        core_ids=list(range(N_CORES)),
        trace=trace,
        **run_kwargs,
    )
    out = np.concatenate([r["out"] for r in res.results], axis=0)
    return out, res


def kernel(**inputs):
    out, _ = _run(inputs, trace=False)
    return out


# revision 16
# speedup vs baseline: 1.1885x; 1.0338x over previous
"""ButterflyMlp Trainium2 kernel.

Reference computation (B=65536):
    h1 = relu(x @ (W1*m1).T + b1)          # [B, 784]
    h2 = relu(h1 @ (W2*m2).T + b2)         # [B, 128]
    logits = h2 @ (W3*m3).T + b3           # [B, 10]
    out = log_softmax(logits, axis=1)

Strategy: pure data parallel over 8 NeuronCores (batch sharded 8192/core,
masked weights replicated).  Activations are kept in transposed
[features, batch] layout on-chip so every layer contracts over the SBUF
partition dimension with the weight tile stationary.  The whole per-core
x shard (8 MB fp8) lives in SBUF, DMA'd in batch-column blocks so the
first block's compute starts while later blocks stream in (DMA
instruction issue costs ~0.6 us each on the queue engine, so few big
transfers beat many small ones).

Layers 1 and 2 run in fp8e4m3 with fp32 PSUM accumulation: the first
768 contraction rows via DoubleRow perf mode (2 fp8 weights per PE cell
-> K=256 per matmul), and the 16-row tail (rows 768..783) via
tile_position row-group packing — the tails of 4 output tiles execute
concurrently in different 32-row groups of the PE array, each
accumulating into its own PSUM bank.  The masked weights are pre-scaled
by 32 (h1 stored at the same x32 scale) to keep fp8 values in the
normal range; the scales are folded back in the relu evacuations, which
alternate between the Scalar and Vector engines.  Layer 3 + log_softmax
run in bf16/fp32.  End-to-end max relative error vs the fp32 reference
is ~3e-4.
"""

import numpy as np
import ml_dtypes

import concourse.bass as bass
import concourse.mybir as mybir
import concourse.tile as tile
from concourse import bacc
from concourse.bass_utils import run_bass_kernel_spmd

BF16 = ml_dtypes.bfloat16
FP8 = ml_dtypes.float8_e4m3
F32 = np.float32

N_CORES = 8
B = 65536
S = B // N_CORES          # batch rows per core
IN_F = 784
KT = 6                    # full 128-row k-tiles (x rows 0..767)
KTAIL = IN_F - KT * 128   # 16-row contraction tail (x rows 768..783)
KT2 = 7                   # h1 feature tiles (896 rows incl. zero padding)
PAD2 = KT2 * 128
H2 = 128
NCLS = 10
NSMX = 16                 # layer-3 batch tiles per softmax group
NGRP = S // (NSMX * 128)  # softmax groups == x DMA blocks
BLKC = S // NGRP          # batch columns per block

SW = 32.0                 # fp8 weight pre-scale (W1, W2); h1 is stored at scale SW

WINDOW, STRIPES, STEP = 10, 5, 3

_CACHE = {}


def _butterfly_mask(out_f, in_f, window=WINDOW, stripes=STRIPES, step=STEP):
    i = np.arange(out_f)[:, None]
    j = np.arange(in_f)[None, :]
    jc = (i * in_f) // out_f
    band = np.abs(j - jc) <= window
    period = max(in_f // stripes, 1)
    stripe = ((j - jc) % period) < step
    return (band | stripe).astype(np.float32)


def _build_nc():
    nc = bacc.Bacc("TRN2", target_bir_lowering=False, debug=False, num_devices=N_CORES)

    # host-side layouts are pre-rearranged so every DMA is contiguous per
    # partition.  xk6 / w1k6 hold the 16-row contraction tail replicated at
    # partition offsets 0/32/64/96 for row-group packing.
    xq = nc.dram_tensor("xq", [KT, 128, S], mybir.dt.float8e4, kind="ExternalInput")
    xk6 = nc.dram_tensor("xk6", [128, S], mybir.dt.float8e4, kind="ExternalInput")
    w1q = nc.dram_tensor("w1q", [128, KT2 * KT * 128], mybir.dt.float8e4, kind="ExternalInput")
    w1k6 = nc.dram_tensor("w1k6", [128, PAD2], mybir.dt.float8e4, kind="ExternalInput")
    w2q = nc.dram_tensor("w2q", [128, KT2 * H2], mybir.dt.float8e4, kind="ExternalInput")
    w3q = nc.dram_tensor("w3q", [H2, NCLS], mybir.dt.bfloat16, kind="ExternalInput")
    bias = nc.dram_tensor("bias", [128, KT2 + 1 + NCLS], mybir.dt.float32, kind="ExternalInput")
    out = nc.dram_tensor("out", [S, NCLS], mybir.dt.float32, kind="ExternalOutput")

    Relu = mybir.ActivationFunctionType.Relu
    Exp = mybir.ActivationFunctionType.Exp
    Ln = mybir.ActivationFunctionType.Ln
    X = mybir.AxisListType.X
    DR = mybir.MatmulPerfMode.DoubleRow

    # output-tile groups whose k-tails run as one packed PE pass
    O_GROUPS = [(0, 1, 2, 3), (4, 5, 6)]

    with tile.TileContext(nc) as tc:
        with (
            tc.tile_pool(name="consts", bufs=1) as consts,
            tc.tile_pool(name="spool", bufs=2) as spool,
            tc.tile_pool(name="ps1", bufs=5, space="PSUM") as ps1,
            tc.tile_pool(name="ps2", bufs=2, space="PSUM") as ps2,
            tc.tile_pool(name="ps3", bufs=1, space="PSUM") as ps3,
        ):
            # w1 SBUF layout [p, o_tile, kt, oi]; single contiguous DMA
            w1_sb = consts.tile([128, KT2, KT, 128], mybir.dt.float8e4)
            nc.sync.dma_start(
                w1_sb[:], w1q.rearrange("p (ot kt oi) -> p ot kt oi", ot=KT2, kt=KT)
            )
            w1k6_sb = consts.tile([128, KT2, 128], mybir.dt.float8e4)
            nc.sync.dma_start(w1k6_sb[:], w1k6.rearrange("p (ot oi) -> p ot oi", ot=KT2))

            # whole x shard in SBUF, first block's columns first
            xt_all = consts.tile([128, KT, S], mybir.dt.float8e4)
            xk6_all = consts.tile([128, S], mybir.dt.float8e4)
            for k in range(KT):
                nc.sync.dma_start(xt_all[:, k, 0:BLKC], xq[k, :, 0:BLKC])
            nc.sync.dma_start(xk6_all[:, 0:BLKC], xk6[:, 0:BLKC])

            w2_sb = consts.tile([128, KT2, H2], mybir.dt.float8e4)
            nc.sync.dma_start(w2_sb[:], w2q.rearrange("p (kt o) -> p kt o", kt=KT2))
            w3_sb = consts.tile([128, NCLS], mybir.dt.bfloat16)
            nc.sync.dma_start(w3_sb[:], w3q[:, :])
            bias_sb = consts.tile([128, KT2 + 1 + NCLS], mybir.dt.float32)
            nc.sync.dma_start(bias_sb[:], bias[:, :])
            b1_sb = bias_sb[:, 0:KT2]
            b2_sb = bias_sb[:, KT2 : KT2 + 1]
            b3_sb = bias_sb[:, KT2 + 1 :]

            for g in range(1, NGRP):
                gs = slice(g * BLKC, (g + 1) * BLKC)
                for k in range(KT):
                    nc.sync.dma_start(xt_all[:, k, gs], xq[k, :, gs])
                nc.sync.dma_start(xk6_all[:, gs], xk6[:, gs])

            # persistent whole-shard activations
            h1_all = consts.tile([128, KT2, S], mybir.dt.float8e4)
            h2_all = consts.tile([128, S], mybir.dt.bfloat16)

            for g in range(NGRP):
                for nbl in range(BLKC // 512):
                    nb = g * (BLKC // 512) + nbl
                    ns = slice(nb * 512, (nb + 1) * 512)

                    # ---- layer 1: h1T = relu(W1mT.T @ xT + b1), fp8 ----
                    for og in O_GROUPS:
                        pss = {}
                        for o in og:
                            ps = ps1.tile([128, 512], mybir.dt.float32, tag="ps1")
                            pss[o] = ps
                            for p in range(3):
                                nc.tensor.matmul(
                                    ps[:],
                                    w1_sb[:, o, 2 * p : 2 * p + 2, :],
                                    xt_all[:, 2 * p : 2 * p + 2, ns],
                                    start=(p == 0),
                                    stop=False,
                                    perf_mode=DR,
                                )
                        # 16-row contraction tails, one packed pass: row
                        # group r serves output tile og[r]
                        for r, o in enumerate(og):
                            nc.tensor.matmul(
                                pss[o][:],
                                w1k6_sb[32 * r : 32 * r + KTAIL, o, :],
                                xk6_all[32 * r : 32 * r + KTAIL, ns],
                                start=False,
                                stop=True,
                                tile_position=(32 * r, 0),
                            )
                        # psum = SW * (x @ W1m.T); h1 stored = relu(psum+SW*b1)
                        # = SW * relu(true + b1).  Evacuations alternate
                        # between the Scalar and Vector engines.
                        for o in og:
                            h1_dst = h1_all[:, o, ns]
                            if o % 2 == 0:
                                nc.vector.tensor_scalar(
                                    h1_dst,
                                    pss[o][:],
                                    b1_sb[:, o : o + 1],
                                    0.0,
                                    mybir.AluOpType.add,
                                    mybir.AluOpType.max,
                                )
                            else:
                                nc.scalar.activation(
                                    h1_dst,
                                    pss[o][:],
                                    Relu,
                                    bias=b1_sb[:, o : o + 1],
                                    scale=1.0,
                                )

                    # ---- layer 2: h2T = relu(W2mT.T @ h1T + b2), fp8 ----
                    ps = ps2.tile([128, 512], mybir.dt.float32, tag="ps2")
                    for p in range(3):
                        nc.tensor.matmul(
                            ps[:],
                            w2_sb[:, 2 * p : 2 * p + 2, :],
                            h1_all[:, 2 * p : 2 * p + 2, ns],
                            start=(p == 0),
                            stop=False,
                            perf_mode=DR,
                        )
                    nc.tensor.matmul(
                        ps[:],
                        w2_sb[:, KT2 - 1, :],
                        h1_all[:, KT2 - 1, ns],
                        start=False,
                        stop=True,
                    )
                    # psum = SW * SW * (h1 @ W2m.T)
                    nc.scalar.activation(
                        h2_all[:, ns],
                        ps[:],
                        Relu,
                        bias=b2_sb[:, 0:1],
                        scale=1.0 / (SW * SW),
                    )

                # ---- layer 3 (bf16): logits[b, c] then log_softmax along c ----
                ps_l = ps3.tile([128, NSMX, NCLS], mybir.dt.float32, tag="ps3")
                for bt in range(NSMX):
                    bt_abs = g * NSMX + bt
                    nc.tensor.matmul(
                        ps_l[:, bt, :],
                        h2_all[:, bt_abs * 128 : (bt_abs + 1) * 128],
                        w3_sb[:, :],
                        start=(bt == 0),
                        stop=(bt == NSMX - 1),
                        skip_group_check=True,
                    )

                z = spool.tile([128, NSMX, NCLS], mybir.dt.float32, tag="z")
                nc.vector.tensor_add(
                    z[:], ps_l[:], b3_sb[:, None, :].to_broadcast((128, NSMX, NCLS))
                )
                zm = spool.tile([128, NSMX], mybir.dt.float32, tag="zm")
                nc.vector.reduce_max(zm[:], z[:], axis=X)
                nc.vector.tensor_sub(
                    z[:], z[:], zm[:, :, None].to_broadcast((128, NSMX, NCLS))
                )
                e = spool.tile([128, NSMX, NCLS], mybir.dt.float32, tag="e")
                nc.scalar.activation(e[:], z[:], Exp)
                se = spool.tile([128, NSMX], mybir.dt.float32, tag="se")
                nc.vector.reduce_sum(se[:], e[:], axis=X)
                lse = spool.tile([128, NSMX], mybir.dt.float32, tag="lse")
                nc.scalar.activation(lse[:], se[:], Ln)
                nc.vector.tensor_sub(
                    e[:], z[:], lse[:, :, None].to_broadcast((128, NSMX, NCLS))
                )
                nc.sync.dma_start(
                    out[g * NSMX * 128 : (g + 1) * NSMX * 128, :].rearrange(
                        "(bt p) c -> p bt c", p=128
                    ),
                    e[:],
                )

    return nc


def _prep_inputs(x, W1, b1, W2, b2, W3, b3):
    m1 = _butterfly_mask(IN_F, IN_F)
    m2 = _butterfly_mask(H2, IN_F)
    m3 = _butterfly_mask(NCLS, H2)

    # w1: [in 784, out(pad 896)] scaled by SW
    w1t = np.zeros((PAD2, PAD2), dtype=F32)
    w1t[:IN_F, :IN_F] = (np.asarray(W1, F32) * m1).T * SW
    # main part: rows 0..767 laid out [p, ot, kt, oi]
    w1l = np.ascontiguousarray(
        w1t[: KT * 128]
        .reshape(KT, 128, KT2, 128)
        .transpose(1, 2, 0, 3)
        .reshape(128, KT2 * KT * 128)
    ).astype(FP8)
    # 16-row tail replicated at partition offsets 0/32/64/96, [p, ot, oi]
    w1k6t = np.zeros((128, KT2, 128), dtype=F32)
    tail = w1t[KT * 128 : KT * 128 + KTAIL].reshape(KTAIL, KT2, 128)
    for r in range(4):
        w1k6t[32 * r : 32 * r + KTAIL] = tail
    w1k6l = np.ascontiguousarray(w1k6t.reshape(128, KT2 * 128)).astype(FP8)

    w2t = np.zeros((PAD2, H2), dtype=F32)
    w2t[:IN_F, :] = (np.asarray(W2, F32) * m2).T * SW
    w2l = np.ascontiguousarray(
        w2t.reshape(KT2, 128, H2).transpose(1, 0, 2).reshape(128, KT2 * H2)
    ).astype(FP8)

    w3l = ((np.asarray(W3, F32) * m3).T).astype(BF16).copy()

    # bias pack [128, 7 + 1 + 10] f32: b1 (x SW, per-partition by o-tile), b2, b3
    b1p = np.zeros((PAD2,), F32)
    b1p[:IN_F] = np.asarray(b1, F32) * SW
    bias = np.zeros((128, KT2 + 1 + NCLS), F32)
    bias[:, 0:KT2] = b1p.reshape(KT2, 128).T
    bias[:, KT2] = np.asarray(b2, F32)
    bias[:, KT2 + 1 :] = np.asarray(b3, F32)[None, :]
    bias = np.ascontiguousarray(bias)

    # x: [B, 784] -> fp8 transposed; main rows 0..767 as [KT, 128, B],
    # tail rows 768..783 replicated at partition offsets 0/32/64/96
    xT = np.asarray(x, F32).T.astype(FP8)
    xp = np.ascontiguousarray(xT[: KT * 128].reshape(KT, 128, B))
    xk6p = np.zeros((128, B), dtype=FP8)
    for r in range(4):
        xk6p[32 * r : 32 * r + KTAIL] = xT[KT * 128 : KT * 128 + KTAIL]

    in_maps = []
    for c in range(N_CORES):
        in_maps.append(
            {
                "xq": np.ascontiguousarray(xp[:, :, c * S : (c + 1) * S]),
                "xk6": np.ascontiguousarray(xk6p[:, c * S : (c + 1) * S]),
                "w1q": w1l,
                "w1k6": w1k6l,
                "w2q": w2l,
                "w3q": w3l,
                "bias": bias,
            }
        )
    return in_maps


def _run(inputs, trace=False, **run_kwargs):
    if "nc" not in _CACHE:
        nc = _build_nc()
        nc.finalize()
        _CACHE["nc"] = nc
    nc = _CACHE["nc"]
    in_maps = _prep_inputs(**inputs)
    res = run_bass_kernel_spmd(
        nc,
        in_maps,
        core_ids=list(range(N_CORES)),
        trace=trace,
        **run_kwargs,
    )
    out = np.concatenate([r["out"] for r in res.results], axis=0)
    return out, res


def kernel(**inputs):
    out, _ = _run(inputs, trace=False)
    return out
